# revision 12
# baseline (speedup 1.0000x reference)
"""Multi-head attention kernel for Trainium2 (8 NeuronCores, SPMD).

Problem: x [4,1,2048,3], W_query/W_key/W_value [1,8,3,3] ->
ctx [4,8,2048,3] = softmax((x Wq)(x Wk)^T / sqrt(3)) @ (x Wv), returned
as a (ctx, ctx) tuple matching the reference.

Sharding: 32 (batch, head) blocks over 8 cores -> core c owns batch c//2,
heads 4*(c%2) .. +4. Each core runs an identical Bass program on its slice.

Per-core program (S=2048, 4 heads in 2 pair-passes, QCH=512, KT=128):
  - Host precomputes Q^T/K^T per head (f32), 2-way bf16 split, stacked as
    9 contract rows per head in row group 32h: scores = q1k1 + q1k2 + q2k1.
  - Scores ring: ONE PSUM tile [128, 3, 1024]; region u%3 holds step u's
    scores^T for the active head pair ([128 k, 512 q] each). Subregion dep
    tracking lets QK(u+1) run during exp(u) with no pool double-buffering.
  - QK: 2 concurrent matmuls (one per head) in distinct 32-row groups
    (contract=9 rows of bf16 stacks), N=512 each.
  - exp: one instruction per step over [128, 1024]. Steps alternate between
    ACT (exact exp, bf16 out) and DVE (Schraudolph: int16 := round(alpha*s
    + beta) whose bits ARE bf16(exp(s/sqrt3)); one fused tensor_scalar).
    This splits the softmax exp - the kernel's dominant cost - across two
    engines that run concurrently.
  - PV: 2 col-group-tiled matmuls (tile_position (0,32j)), lhsT =
    [x_bf16 | ones] [128,4], accumulating [4,512] ctx+denom per head; each
    head's group owns its own PSUM bank (zero-region rule).
  - Per (pass, qc) boundary: copy ctx to SBUF, PE transpose-and-fold via
    wv4 into ctx bank 0 (legal between group stop and next start), DVE
    reciprocal + broadcast multiply, DMA out.
"""

import math

import numpy as np
import ml_dtypes

import concourse.bass as bass
import concourse.bacc as bacc
import concourse.tile as tile
from concourse import mybir
from concourse.bass_utils import run_bass_kernel_spmd

f32 = mybir.dt.float32
bf16 = mybir.dt.bfloat16
i16 = mybir.dt.int16
EXP = mybir.ActivationFunctionType.Exp
MUL = mybir.AluOpType.mult
ADD = mybir.AluOpType.add

B, H, S, D = 4, 8, 2048, 3
NCORES = 8
HPC = H // 2           # heads per core = 4
QCH = 512              # query chunk
NQ = S // QCH          # 4
KT = 128               # key tile
NKT = S // KT          # 16
NU = 2 * NQ * NKT      # 128 (pass, qc, t) steps
SCALE = 1.0 / math.sqrt(D)

# Schraudolph exp for bf16: bits(bf16(e^(SCALE*s))) ~ round(ALPHA*s + BETA)
ALPHA = SCALE * math.log2(math.e) * 128.0
BETA = 127.0 * 128.0 - 5.45
# steps (by t = u % NKT) whose exp runs on DVE; rest on ACT
DVE_T = frozenset((1, 3, 5, 7, 9, 11, 13))


def _build_nc():
    nc = bacc.Bacc("TRN2", target_bir_lowering=False, debug=False,
                   num_devices=NCORES)

    qstk_in = nc.dram_tensor("qstk", [128, S], bf16, kind="ExternalInput").ap()
    kstk_in = nc.dram_tensor("kstk", [128, S], bf16, kind="ExternalInput").ap()
    xov_in = nc.dram_tensor("xov", [128, NKT, 4], bf16,
                            kind="ExternalInput").ap()
    wv4_in = nc.dram_tensor("wv4", [128, 2, 2, 4], f32,
                            kind="ExternalInput").ap()
    out = nc.dram_tensor("out", [HPC, S, D], f32, kind="ExternalOutput").ap()

    with tile.TileContext(nc) as tc:
        with tc.tile_pool(name="persist", bufs=1) as per, \
             tc.tile_pool(name="work", bufs=1) as work, \
             tc.tile_pool(name="score_ps", bufs=1, space="PSUM") as sps, \
             tc.tile_pool(name="acc_ps", bufs=1, space="PSUM") as aps:
            qstk = per.tile([128, S], bf16)
            kstk = per.tile([128, S], bf16)
            xov = per.tile([128, NKT, 4], bf16)
            wv4 = per.tile([128, 2, 2, 4], f32)
            nc.sync.dma_start(out=qstk, in_=qstk_in)
            nc.gpsimd.dma_start(out=kstk, in_=kstk_in)
            nc.sync.dma_start(out=xov, in_=xov_in)
            nc.gpsimd.dma_start(out=wv4, in_=wv4_in)

            ctx_ps = aps.tile([128, 2, QCH], f32)  # 2 banks: 1 per head

            st = {}               # in-flight score tiles (u -> psum tile)

            def emit_qk(u):
                pp, r0 = divmod(u, NQ * NKT)
                qc, t = divmod(r0, NKT)
                cs = slice(qc * QCH, (qc + 1) * QCH)
                # ring of 3 score tiles: separate tiles (not subregions of
                # one tile) so Tile's tensor-granular hazard tracking stays
                # exact and QK(u+1)/exp(u+1) overlap exp(u)/exp(u-1)
                s = sps.tile([128, 2, QCH], f32, name=f"s{u}", tag="s",
                             bufs=3)
                st[u] = s
                for j in range(2):
                    g = 32 * (2 * pp + j)
                    nc.tensor.matmul(
                        s[:, j, :],
                        lhsT=kstk[g:g + 9, t * KT:(t + 1) * KT],
                        rhs=qstk[g:g + 9, cs],
                        start=True, stop=True,
                        tile_position=(g, 0),
                    )

            def norm_pieces(pp, qc):
                ctx_sb = work.tile([128, 2 * QCH], f32, name=f"cs{pp}{qc}",
                                   tag="cs", bufs=2)
                rec = work.tile([128, 8], f32, name=f"rec{pp}{qc}",
                                tag="rec", bufs=2)
                ostage = work.tile([128, 8, 3], f32, name=f"o{pp}{qc}",
                                   tag="o", bufs=2)
                b0 = ctx_ps[:, 0, :]

                def p_copy():
                    for j in range(2):
                        nc.vector.tensor_copy(
                            ctx_sb[32 * j:32 * j + 4, j * QCH:(j + 1) * QCH],
                            ctx_ps[32 * j:32 * j + 4, j, :])

                # transpose-and-fold: ct[q, 4] = [ctx_un(3) | den] per
                # (head j, 128-q chunk c), written into ctx bank 0 slots
                # (its accumulation group just stopped; next starts later).
                # One accumulation group over all 8 disjoint subregions:
                # each start=True would lazily zero the WHOLE bank.
                def mk_ct(jj):
                    def go():
                        for j in jj:
                            for c in range(4):
                                sl = 4 * j + c
                                nc.tensor.matmul(
                                    b0[:, 4 * sl:4 * sl + 4],
                                    lhsT=ctx_sb[32 * j:32 * j + 4,
                                                j * QCH + c * KT:
                                                j * QCH + (c + 1) * KT],
                                    rhs=wv4[32 * j:32 * j + 4, pp, j, :],
                                    start=(sl == 0), stop=(sl == 7),
                                    tile_position=(32 * j, 0),
                                )
                    return go

                def p_recmul():
                    den = bass.AP(tensor=b0.tensor, offset=b0.offset + 3,
                                  ap=[list(b0.ap[0]), [4, 8], [1, 1]])
                    un = bass.AP(tensor=b0.tensor, offset=b0.offset,
                                 ap=[list(b0.ap[0]), [4, 8], [1, 3]])
                    nc.vector.reciprocal(rec, den)
                    rb = rec[:, :].unsqueeze(2).broadcast_to([128, 8, 3])
                    nc.vector.tensor_mul(ostage, un, rb)

                def p_out():
                    for j in range(2):
                        dst = bass.AP(
                            tensor=out.tensor,
                            offset=(2 * pp + j) * S * D + qc * QCH * D,
                            ap=[[D, 128], [KT * D, 4], [1, D]],
                        )
                        nc.sync.dma_start(out=dst,
                                          in_=ostage[:, 4 * j:4 * j + 4, :])

                return [p_copy, mk_ct((0,)), mk_ct((1,)), p_recmul, p_out]

            pend = []      # boundary normalize pieces, dripped 2/step
            pv_hold = []   # PV emissions held until the pieces drain

            emit_qk(0)
            emit_qk(1)
            for u in range(NU):
                pp, r0 = divmod(u, NQ * NKT)
                qc, t = divmod(r0, NKT)
                s = st.pop(u)
                # separate tag rings per producing engine: a shared tag
                # chains allocations in order, serializing ACT and DVE exps
                if t in DVE_T:
                    p = work.tile([128, 1024], bf16, name=f"p{u}", tag="pd",
                                  bufs=4)
                    nc.vector.tensor_scalar(
                        p[:, :].bitcast(i16), s[:, :, :],
                        ALPHA, BETA, MUL, ADD)
                else:
                    p = work.tile([128, 1024], bf16, name=f"p{u}", tag="pa",
                                  bufs=4)
                    nc.scalar.activation(p, s[:, :, :], EXP, scale=SCALE)
                if u + 2 < NU:
                    emit_qk(u + 2)
                for _ in range(2):
                    if pend:
                        pend.pop(0)()

                def emit_pv(_t=t, _p=p):
                    for j in range(2):
                        nc.tensor.matmul(
                            ctx_ps[32 * j:32 * j + 4, j, :],
                            lhsT=xov[:, _t, :],
                            rhs=_p[:, j * QCH:(j + 1) * QCH],
                            start=(_t == 0), stop=(_t == NKT - 1),
                            tile_position=(0, 32 * j),
                        )

                if pend:
                    pv_hold.append(emit_pv)
                else:
                    while pv_hold:
                        pv_hold.pop(0)()
                    emit_pv()
                if t == NKT - 1:
                    pend = norm_pieces(pp, qc)
            while pend:
                pend.pop(0)()
            while pv_hold:
                pv_hold.pop(0)()

    nc.compile()
    return nc


_NC_CACHE = None


def _get_nc():
    global _NC_CACHE
    if _NC_CACHE is None:
        _NC_CACHE = _build_nc()
    return _NC_CACHE


def _bf(a):
    return a.astype(ml_dtypes.bfloat16)


def _make_in_maps(x, W_query, W_key, W_value):
    in_maps = []
    for c in range(NCORES):
        b = c // 2
        hp = (c % 2) * HPC
        xb = np.ascontiguousarray(x[b, 0])              # [S, 3] f32

        qstk = np.zeros((128, S), ml_dtypes.bfloat16)
        kstk = np.zeros((128, S), ml_dtypes.bfloat16)
        for g in range(HPC):
            Qt = np.ascontiguousarray((xb @ W_query[0, hp + g]).T)  # [3, S]
            Kt = np.ascontiguousarray((xb @ W_key[0, hp + g]).T)
            q1 = _bf(Qt)
            q2 = _bf(Qt - q1.astype(np.float32))
            k1 = _bf(Kt)
            k2 = _bf(Kt - k1.astype(np.float32))
            # terms: q1*k1 + q1*k2 + q2*k1
            qstk[32 * g + 0:32 * g + 3] = q1
            qstk[32 * g + 3:32 * g + 6] = q1
            qstk[32 * g + 6:32 * g + 9] = q2
            kstk[32 * g + 0:32 * g + 3] = k1
            kstk[32 * g + 3:32 * g + 6] = k2
            kstk[32 * g + 6:32 * g + 9] = k1

        # xov[p, t, :] = [x_bf16(t*128+p, 0:3) | 1]
        xov = np.concatenate([xb, np.ones((S, 1), np.float32)], axis=1)
        xov = _bf(xov.reshape(NKT, KT, 4).transpose(1, 0, 2))
        xov = np.ascontiguousarray(xov)

        # wv4[32j+d, pp, j, e] = Wv[2pp+j][d, e]; [32j+3, pp, j, 3] = 1
        wv4 = np.zeros((128, 2, 2, 4), np.float32)
        for pp in range(2):
            for j in range(2):
                wv4[32 * j:32 * j + 3, pp, j, 0:3] = W_value[0, hp + 2 * pp + j]
                wv4[32 * j + 3, pp, j, 3] = 1.0

        in_maps.append({
            "qstk": qstk,
            "kstk": kstk,
            "xov": xov,
            "wv4": wv4,
        })
    return in_maps


def kernel(x, W_query, W_key, W_value, _trace=False, _tmpdir=None):
    x = np.asarray(x, dtype=np.float32)
    W_query = np.asarray(W_query, dtype=np.float32)
    W_key = np.asarray(W_key, dtype=np.float32)
    W_value = np.asarray(W_value, dtype=np.float32)

    nc = _get_nc()
    res = run_bass_kernel_spmd(
        nc,
        _make_in_maps(x, W_query, W_key, W_value),
        core_ids=list(range(NCORES)),
        trace=_trace,
        tmpdir=_tmpdir,
    )
    full = np.empty((B, H, S, D), dtype=np.float32)
    for c in range(NCORES):
        b = c // 2
        hp = (c % 2) * HPC
        full[b, hp:hp + HPC] = res.results[c]["out"]
    if _trace:
        kernel._last_results = res
    return (full, full)
